# revision 4
# baseline (speedup 1.0000x reference)
"""MoE routing gate kernel for Trainium2 (8 NeuronCores, data-parallel over tokens).

Computes, for x [16384, 2048] f32 and weight [64, 2048] f32:
    logits = x @ weight.T            # [16384, 64]
    scores = softmax(logits)
    vals, idx = top_k(scores, 2)
    vals = vals / vals.sum(-1, keepdims=True)
    returns (vals f32 [16384,2], idx int32 [16384,2])

Key identity: the softmax denominator cancels in the renormalized top-2:
    v1 = 1/(1+e), v2 = e/(1+e) with e = exp(l2 - l1), l1 >= l2 the top-2 logits.
So per token we only need the top-2 logits and their indices.

Per-core dataflow (2048 tokens):
  - slab loop (512 tokens): DMA x rows naturally [128tok, 2048d];
    PE-transpose 128x128 blocks into PSUM; copy PSUM->SBUF (DVE/ACT split)
    to build xT [128d, 512tok] per k-chunk; matmul with wT stationary
    (K=128d, M=64e, N=512tok) accumulating logits PSUM [64, 512];
    transpose logits back to [128tok, 64e]; top-2 via DVE max/max_index;
    values via exp + reciprocal; DMA out per token-tile.
"""

import numpy as np

import concourse.bacc as bacc
import concourse.mybir as mybir
from concourse.bass_utils import run_bass_kernel_spmd
from concourse.masks import make_identity
from concourse.tile import TileContext

# Problem constants (hardcoded per contract)
N_CORES = 8
T_FULL = 16384
D = 2048
E = 64
P = 128
T = T_FULL // N_CORES      # 2048 tokens per core
KC = D // P                # 16 contraction chunks
SLAB = 512                 # tokens per slab
NSLAB = T // SLAB          # 4
TPS = SLAB // P            # 4 token-tiles per slab

F32 = mybir.dt.float32
I32 = mybir.dt.int32
U32 = mybir.dt.uint32

_compiled = {}


def _build():
    nc = bacc.Bacc(
        "TRN2",
        target_bir_lowering=False,
        debug=False,
        enable_asserts=False,
        num_devices=N_CORES,
    )
    x = nc.declare_dram_parameter("x", [T, D], F32, isOutput=False)
    w = nc.declare_dram_parameter("weight", [E, D], F32, isOutput=False)
    out_v = nc.declare_dram_parameter("values", [T, 2], F32, isOutput=True)
    out_i = nc.declare_dram_parameter("indices", [T, 2], I32, isOutput=True)

    with TileContext(nc) as tc:
        with (
            tc.tile_pool(name="const", bufs=1) as const_pool,
            tc.tile_pool(name="xnat", bufs=6) as x_pool,
            tc.tile_pool(name="xt", bufs=2) as xt_pool,
            tc.tile_pool(name="small", bufs=4) as small_pool,
            tc.tile_pool(name="tpsum", bufs=4, space="PSUM") as tpsum_pool,
            tc.tile_pool(name="lpsum", bufs=2, space="PSUM") as lpsum_pool,
            tc.tile_pool(name="ltpsum", bufs=2, space="PSUM") as ltpsum_pool,
        ):
            # ---- one-time setup: identity + transposed weight ----
            ident = const_pool.tile([P, P], F32)
            make_identity(nc, ident)

            w_sb = const_pool.tile([E, D], F32)
            nc.sync.dma_start(out=w_sb, in_=w[:, :])

            # wT [128d, KC, 64e]
            wT = const_pool.tile([P, KC, E], F32)
            for j in range(KC // 4):
                pt = tpsum_pool.tile([P, 4, E], F32, tag="tp")
                for c in range(4):
                    k = 4 * j + c
                    nc.tensor.transpose(
                        out=pt[:, c, :],
                        in_=w_sb[:, k * P:(k + 1) * P],
                        identity=ident[:E, :E],
                    )
                nc.vector.tensor_copy(out=wT[:, 4 * j:4 * j + 4, :], in_=pt)

            # ---- main loop over slabs of 512 tokens ----
            copy_rr = 0  # round-robin DVE/ACT for the big PSUM->SBUF copies
            for s in range(NSLAB):
                xt = xt_pool.tile([P, KC, SLAB], F32)  # 32KB/partition
                for t in range(TPS):
                    row0 = (s * TPS + t) * P
                    xn = x_pool.tile([P, D], F32)
                    nc.sync.dma_start(out=xn, in_=x[row0:row0 + P, :])
                    for j in range(KC // 4):
                        pt = tpsum_pool.tile([P, 4, P], F32, tag="tp")
                        for c in range(4):
                            k = 4 * j + c
                            nc.tensor.transpose(
                                out=pt[:, c, :],
                                in_=xn[:, k * P:(k + 1) * P],
                                identity=ident,
                            )
                        dst = xt[:, 4 * j:4 * j + 4, t * P:(t + 1) * P]
                        if copy_rr % 3 == 2:
                            nc.scalar.copy(out=dst, in_=pt)
                        else:
                            nc.vector.tensor_copy(out=dst, in_=pt)
                        copy_rr += 1

                # logits^T [64e, 512tok] accumulated over 16 k-chunks
                lp = lpsum_pool.tile([E, SLAB], F32)
                for k in range(KC):
                    nc.tensor.matmul(
                        out=lp,
                        lhsT=wT[:, k, :],
                        rhs=xt[:, k, :],
                        start=(k == 0),
                        stop=(k == KC - 1),
                    )
                l_sb = small_pool.tile([E, SLAB], F32, tag="lsb")
                nc.vector.tensor_copy(out=l_sb, in_=lp)

                # transpose logits back: [64, 512] -> 4x [128, 64]
                ltp = ltpsum_pool.tile([P, TPS, E], F32)
                for c in range(TPS):
                    nc.tensor.transpose(
                        out=ltp[:, c, :],
                        in_=l_sb[:, c * P:(c + 1) * P],
                        identity=ident[:E, :E],
                    )
                lt = small_pool.tile([P, TPS, E], F32, tag="lt")
                nc.vector.tensor_copy(out=lt, in_=ltp)

                # top-2 per token-tile
                m8 = small_pool.tile([P, TPS, 8], F32, tag="m8")
                i8 = small_pool.tile([P, TPS, 8], U32, tag="i8")
                for c in range(TPS):
                    nc.vector.max(out=m8[:, c, :], in_=lt[:, c, :])
                    nc.vector.max_index(
                        out=i8[:, c, :], in_max=m8[:, c, :], in_values=lt[:, c, :]
                    )

                # values: v1 = 1/(1+e), v2 = e/(1+e), e = exp(l2 - l1)
                vals = small_pool.tile([P, TPS, 2], F32, tag="vals")
                idxs = small_pool.tile([P, TPS, 2], I32, tag="idxs")
                dt_ = small_pool.tile([P, TPS], F32, tag="dt")
                et = small_pool.tile([P, TPS], F32, tag="et")
                dn = small_pool.tile([P, TPS], F32, tag="dn")
                nc.vector.tensor_sub(out=dt_, in0=m8[:, :, 1], in1=m8[:, :, 0])
                nc.scalar.activation(
                    out=et, in_=dt_, func=mybir.ActivationFunctionType.Exp
                )
                nc.vector.tensor_scalar_add(dn, et, 1.0)
                nc.vector.reciprocal(out=vals[:, :, 0], in_=dn)
                nc.vector.tensor_mul(out=vals[:, :, 1], in0=et, in1=vals[:, :, 0])
                nc.vector.tensor_copy(out=idxs, in_=i8[:, :, 0:2])

                # store
                for t in range(TPS):
                    row0 = (s * TPS + t) * P
                    nc.sync.dma_start(out=out_v[row0:row0 + P, :], in_=vals[:, t, :])
                    nc.sync.dma_start(out=out_i[row0:row0 + P, :], in_=idxs[:, t, :])

    nc.compile()
    return nc


def _get_nc():
    if "nc" not in _compiled:
        _compiled["nc"] = _build()
    return _compiled["nc"]


def kernel(x: np.ndarray, weight: np.ndarray):
    x = np.ascontiguousarray(x, dtype=np.float32)
    weight = np.ascontiguousarray(weight, dtype=np.float32)
    nc = _get_nc()
    in_maps = [
        {"x": x[i * T:(i + 1) * T], "weight": weight} for i in range(N_CORES)
    ]
    res = run_bass_kernel_spmd(nc, in_maps, list(range(N_CORES)))
    values = np.concatenate([r["values"] for r in res.results], axis=0)
    indices = np.concatenate([r["indices"] for r in res.results], axis=0)
    return values, indices
